# revision 46
# baseline (speedup 1.0000x reference)
"""NodeRoIPool Trainium2 kernel.

For each of 20000 ROIs (8 corner coords), 5 points (4 edge midpoints +
centroid) are snapped to the feature grid (ceil, clip to [2,254]) and a
4x4 window of feat [256,256,256] is mean-pooled across all 256 channels,
giving out [20000, 1280] (point-major, channel-fastest).

Algorithm: the 4x4 mean only depends on the snapped point, so compute a
4x4 box-filtered feature map once (separable DVE shift-adds in bf16 on
feat pre-scaled by 1/16 on the host), PE-transpose it to channel-last
rows, and turn each point into a single 1KB super-row gather (4
consecutive y-cells; the host picks the point's 256B slice), which cuts
the gpsimd descriptor count ~30% via denser dedup.

Sharding (8 cores): 2-way channel x 4-way ROI (5000 ROIs x 128 channels
per core).  The DVE filter chain and the gpsimd gather stream are the
two time poles; halving channels halves DVE work while the gathers run
concurrently on gpsimd.  The box-filtered map is written to EIGHT DRAM
tensors (32 y-rows each, [8192 rows x 128ch] bf16) in an (xl, y, xb, c)
layout:
  - the per-chunk store is 4KB contiguous per partition (no packet storm)
  - super-row ids fit int16, so points gather single 1KB rows
  - the gather of sub-band s starts as soon as its 32 y-rows are
    filtered, overlapping the rest of the filter
Gather row indices are computed on the HOST from rois (bit-identical
fp32 op order to the reference), deduplicated and sorted per sub-band;
per-call gather sizes are baked statically (max over cores, rounded up
to 128).  The host expands/unpermutes the gathered bf16 rows into the
final [20000, 1280] fp32 output.
"""

import numpy as np

import concourse.bass as bass
import concourse.tile as tile
from concourse import bacc, mybir
from concourse import bass_utils
from concourse.masks import make_identity

N_CORES = 8
CH_SHARD = 2
ROI_SHARD = 4
C, H, W = 256, 256, 256
CS = C // CH_SHARD          # 128 channels per core
N_ROIS = 20000
RPC = N_ROIS // ROI_SHARD   # 5000 rois per core
PPC = RPC * 5               # 25000 points per core
NBANDS = 8                  # y sub-bands of 32 rows, one box tensor each
YB = H // NBANDS            # 32
YCHUNK = 16
BF16 = mybir.dt.bfloat16
I16 = mybir.dt.int16

_prog_cache = {}


def _build_program(caps, stages=("filter", "gather")):
    """caps: per-sub-band static gather counts (each %128 == 0)."""
    assert len(caps) == NBANDS and all(c % 128 == 0 for c in caps)
    slots = [c // 128 for c in caps]
    off_slots = np.concatenate([[0], np.cumsum(slots)])
    tot_slots = int(off_slots[-1])
    idxw = max(c // 16 for c in caps)

    nc = bacc.Bacc("TRN2", target_bir_lowering=False, debug=False,
                   num_devices=N_CORES,
                   # SWDGE descriptor-ring carveout: ring capacity is
                   # size//16 descs; the default 1024 blocks gather
                   # desc-gen mid-filter (ring-full, drain starved)
                   dynamic_dma_scratch_size=49152)

    feat_in = nc.dram_tensor("feat", [CS, H, W], BF16, kind="ExternalInput")
    idx_in = nc.dram_tensor("idx", [NBANDS, 128, idxw], I16,
                            kind="ExternalInput")
    out_t = nc.dram_tensor("out", [128 * tot_slots, 4 * CS], BF16,
                           kind="ExternalOutput")
    boxes = [nc.dram_tensor(f"box{s}", [128 * 2 * YB, CS], BF16,
                            kind="Internal") for s in range(NBANDS)]

    with tile.TileContext(nc) as tc:
        with (
            tc.tile_pool(name="sbuf", bufs=1) as pool,
            tc.tile_pool(name="dve", bufs=2) as dp,
            # bufs=2: fin loads are gated just-in-time by the DVE consuming
            # two chunks back, so the scalar HWDGE ring empties between
            # loads -- the strictly-lower-priority software (gather) queue
            # only gets SDMA service in those windows
            tc.tile_pool(name="fin", bufs=2) as fp,
            tc.tile_pool(name="vvp", bufs=3) as vp,
            tc.tile_pool(name="stgp", bufs=3) as sp,
            tc.tile_pool(name="gather", bufs=3) as gp,
            tc.tile_pool(name="psum", bufs=2, space="PSUM") as pp,
        ):
            ident = pool.tile([128, 128], BF16, tag="ident")
            make_identity(nc, ident[:])

            idx_t = pool.tile([128, NBANDS, idxw], I16, tag="idx")
            nc.sync.dma_start(
                out=idx_t[:], in_=idx_in.rearrange("c p s -> p c s"))

            views = [b.rearrange("(xb xl y) c -> xl xb y c", xb=2, y=YB)
                     for b in boxes]

            # y rows never produced by the filter (y'<2 in band 0, y'=255
            # in band 7): zero-fill so stray reads in simulation stay
            # finite.  x' in {0,1,255} is zeroed in vv.
            zt = pool.tile([128, 2, 2, CS], BF16, tag="zt")
            nc.vector.memset(zt[:], 0.0)
            nc.sync.dma_start(out=views[0][:, :, 0:2, :], in_=zt[:])
            nc.sync.dma_start(out=views[-1][:, :, YB - 1:YB, :],
                              in_=zt[:, :, 0:1])

            # warmup gather at t~0: pays the Q7 gather-library load and
            # first-call cold cost (~4us) off the critical path, gathering
            # garbage from feat into scratch
            widx = pool.tile([128, 8], I16, tag="widx")
            nc.vector.memset(widx[:], 0)
            wout = pool.tile([128, 1, CS], BF16, tag="wout")
            feat2d = feat_in.rearrange("c h (x k) -> (c h x) k", k=CS)
            nc.gpsimd.dma_gather(
                wout[:], feat2d, widx[:], 128, 128, CS, single_packet=False)

            ov = out_t.rearrange("(p s) ch -> p s ch", s=tot_slots)

            pending_out = []

            def emit_out():
                # on sync, forced to the end of its stream via the
                # scheduler-sim readiness gate: out drains then run on q1
                # in parallel with q0's gather drains, and the gather
                # chain on gpsimd carries no out-wait stalls at all
                if pending_out:
                    s, gt = pending_out.pop(0)
                    s0 = int(off_slots[s])
                    with tc.tile_wait_until(0.2):
                        nc.sync.dma_start(
                            out=ov[:, s0:s0 + slots[s], :],
                            in_=gt[:, 0:slots[s], :])

            def emit_gather(s):
                # gathers 1KB super-rows (4 consecutive y-cells); the host
                # picks each point's 256B slice.  gt streams band by band;
                # each band's out-DMA is emitted AFTER the NEXT band's
                # gather (both on gpsimd), so the out's drain-wait is
                # covered by that gather's desc-gen instead of gapping the
                # gather chain.  q0's FIFO keeps out-before-regather order.
                if "gather" not in stages:
                    return
                gt = gp.tile([128, max(slots), 4 * CS], BF16, tag="gt")
                nc.gpsimd.dma_gather(
                    gt[:, 0:slots[s], :],
                    boxes[s].rearrange("(r q) c -> r (q c)", q=4),
                    idx_t[:, s, 0:caps[s] // 16],
                    caps[s],
                    caps[s],
                    4 * CS,
                    single_packet=False,
                )
                pending_out.append((s, gt))

            # ---------------- box filter ---------------------------------
            # 4x4 box mean, windows [i-2, i+1] both axes; host pre-divided
            # feat by 16 so no scaling on device.  The two chunks of each
            # band are software-pipelined (DVE ops interleaved) so the
            # ~2.5us semaphore-propagation latency between dependent
            # same-engine ops is hidden behind the sibling chunk's op.

            # the vv tiles' edge columns (x' in {0,1,255}) are zeroed once
            # per buffer instance and never overwritten by the filter ops
            for _ in range(3):
                vv0 = vp.tile([128, YCHUNK, W], BF16, tag="vv")
                nc.vector.memset(vv0[:, :, 0:2], 0.0)
                nc.vector.memset(vv0[:, :, W - 1:W], 0.0)

            def chunk_params(ci):
                a = max(2, ci * YCHUNK)
                b = min(H - 1, (ci + 1) * YCHUNK)
                ys0 = a - 2
                ys1 = min(H, b + 1)
                return a, b - a, ys0, ys1 - ys0

            def dve_steps(ci):
                a, nv, ys0, nr = chunk_params(ci)
                fin = fp.tile([128, YCHUNK + 3, W], BF16, tag="fin")
                nc.scalar.dma_start(
                    out=fin[:, 0:nr, :], in_=feat_in[:, ys0:ys0 + nr, :])
                s1 = dp.tile([128, YCHUNK + 3, W - 1], BF16, tag="s1")
                hh = dp.tile([128, YCHUNK + 3, W], BF16, tag="hh")
                uu = dp.tile([128, YCHUNK + 2, W], BF16, tag="uu")
                vv = vp.tile([128, YCHUNK, W], BF16, tag="vv")
                yield lambda: nc.vector.tensor_tensor(
                    out=s1[:, 0:nr, :], in0=fin[:, 0:nr, 0:W - 1],
                    in1=fin[:, 0:nr, 1:W], op=mybir.AluOpType.add)
                yield lambda: nc.vector.tensor_tensor(
                    out=hh[:, 0:nr, 2:W - 1], in0=s1[:, 0:nr, 0:W - 3],
                    in1=s1[:, 0:nr, 2:W - 1], op=mybir.AluOpType.add)
                yield lambda: nc.vector.tensor_tensor(
                    out=uu[:, 0:nr - 1, 2:W - 1],
                    in0=hh[:, 0:nr - 1, 2:W - 1],
                    in1=hh[:, 1:nr, 2:W - 1], op=mybir.AluOpType.add)
                o0 = a - 2 - ys0
                o1 = a - ys0
                yield lambda: nc.vector.tensor_tensor(
                    out=vv[:, 0:nv, 2:W - 1],
                    in0=uu[:, o0:o0 + nv, 2:W - 1],
                    in1=uu[:, o1:o1 + nv, 2:W - 1],
                    op=mybir.AluOpType.add)
                yield vv

            def back_half(ci, vv):
                a, nv, ys0, nr = chunk_params(ci)
                sb = ci // 2
                # transpose [c, x128] -> [x128, c]; one matmul <= one bank
                stg = sp.tile([128, 2, YCHUNK, CS], BF16, tag="stg")
                for xb in range(2):
                    for g0 in range(0, nv, 4):
                        gn = min(4, nv - g0)
                        pt = pp.tile([128, 4, 1024], BF16, tag="tp")
                        for j in range(gn):
                            nc.tensor.transpose(
                                out=pt[:, j, 0:CS],
                                in_=vv[:, g0 + j, xb * 128:(xb + 1) * 128],
                                identity=ident[:],
                            )
                        nc.scalar.activation(
                            out=stg[:, xb, g0:g0 + gn, :],
                            in_=pt[:, 0:gn, 0:CS],
                            func=mybir.ActivationFunctionType.Copy,
                            scale=1.0,
                        )
                nc.sync.dma_start(
                    out=views[sb][:, :, a - YB * sb:a - YB * sb + nv, :],
                    in_=stg[:, :, 0:nv, :],
                )

            if "filter" in stages:
                # big-gather bands first so gpsimd starts its long work
                # early; the last bands (0, 7) leave only small gather
                # tails, and the bulk of the output DMA (bands 1..6, a
                # contiguous slot range) overlaps those final gathers
                order = sorted(range(NBANDS), key=lambda s: -caps[s])
                for k, sb in enumerate(order):
                    g0 = dve_steps(2 * sb)
                    g1 = dve_steps(2 * sb + 1)
                    for st0, st1 in zip(g0, g1):
                        if callable(st0):
                            st0()
                            st1()
                        else:
                            back_half(2 * sb, st0)
                            back_half(2 * sb + 1, st1)
                    emit_gather(sb)
                    if len(pending_out) >= 3:
                        emit_out()
                emit_out()
                emit_out()
                emit_out()
            else:
                for s in range(NBANDS):
                    emit_gather(s)
                    emit_out()
                emit_out()

    nc.compile()
    return nc


def _host_rows(rois_core: np.ndarray):
    """Sub-band + box-row index of each of one core's 25000 points.

    Replicates the reference's fp32 op order exactly (matches the jax cpu
    result bit-for-bit, so ceil never flips vs the oracle).
    """
    f32 = np.float32
    q = (rois_core.astype(f32) * f32(0.25)).reshape(-1, 4, 2)
    mids = (q + np.roll(q, -1, axis=1)) * f32(0.5)
    csum = ((q[:, 0] + q[:, 1]) + q[:, 2]) + q[:, 3]
    center = csum * f32(0.25)
    pts = np.concatenate([mids, center[:, None, :]], axis=1)  # [N,5,2]
    xc = np.clip(np.ceil(pts[..., 0]), 2.0, 254.0).astype(np.int64)
    yc = np.clip(np.ceil(pts[..., 1]), 2.0, 254.0).astype(np.int64)
    band = yc // YB
    yl = yc % YB
    # 1KB super-row r holds cells (x, 4*(yl//4) .. +3); j picks the cell
    row = xc * (YB // 4) + yl // 4
    return band.ravel(), row.ravel(), (yl % 4).ravel()


def kernel(feat: np.ndarray, rois: np.ndarray) -> np.ndarray:
    feat = np.asarray(feat, dtype=np.float32)
    rois = np.ascontiguousarray(np.asarray(rois, dtype=np.float32))
    assert feat.shape == (C, H, W) and rois.shape == (N_ROIS, 8)

    # unique sorted box rows per (roi-shard, sub-band)
    per_shard = []
    counts = np.zeros((ROI_SHARD, NBANDS), np.int64)
    for ri in range(ROI_SHARD):
        band, row, jsl = _host_rows(rois[ri * RPC:(ri + 1) * RPC])
        uniqs = []
        invs = []
        for s in range(NBANDS):
            sel = band == s
            uniq, inv = np.unique(row[sel], return_inverse=True)
            counts[ri, s] = len(uniq)
            uniqs.append(uniq)
            invs.append(inv)
        per_shard.append((band, uniqs, invs, jsl))

    caps = tuple(int(-(-int(counts[:, s].max() + 1) // 128) * 128)
                 for s in range(NBANDS))
    if caps not in _prog_cache:
        _prog_cache[caps] = _build_program(caps)
    nc = _prog_cache[caps]

    slots = [cp // 128 for cp in caps]
    off_slots = np.concatenate([[0], np.cumsum(slots)])
    tot_slots = int(off_slots[-1])
    idxw = max(cp // 16 for cp in caps)

    bf16 = mybir.dt.np(BF16)
    fb = np.ascontiguousarray((feat * np.float32(1.0 / 16.0)).astype(bf16))

    # idx layout + output row of each point, per ROI shard (shared by the
    # two channel-shard cores)
    idx_maps = []
    dram_rows = []
    jsls = []
    for ri in range(ROI_SHARD):
        band, uniqs, invs, jsl = per_shard[ri]
        jsls.append(jsl)
        # pad with row 0 (valid): negative "ignored" indices trip an OOB
        # DMA address on hardware
        idx = np.zeros((NBANDS, 16, idxw), np.int16)
        dram_row = np.empty(PPC, np.int64)
        for s in range(NBANDS):
            uniq, inv = uniqs[s], invs[s]
            nu = len(uniq)
            assert nu <= caps[s]
            i = np.arange(nu)
            idx[s, i % 16, i // 16] = uniq.astype(np.int16)
            st = (i % 128) * tot_slots + off_slots[s] + i // 128
            dram_row[band == s] = st[inv]
        idx_maps.append(np.ascontiguousarray(np.tile(idx, (1, 8, 1))))
        dram_rows.append(dram_row)

    in_maps = []
    for core in range(N_CORES):
        ci, ri = divmod(core, ROI_SHARD)
        in_maps.append({
            "feat": np.ascontiguousarray(fb[ci * CS:(ci + 1) * CS]),
            "idx": idx_maps[ri],
        })

    res = bass_utils.run_bass_kernel_spmd(
        nc, in_maps, core_ids=list(range(N_CORES)))

    out = np.empty((ROI_SHARD, RPC, 5, CH_SHARD, CS), dtype=np.float32)
    pick = np.arange(PPC)
    for core in range(N_CORES):
        ci, ri = divmod(core, ROI_SHARD)
        vals = np.asarray(res.results[core]["out"])[dram_rows[ri]]
        vals = vals.reshape(PPC, 4, CS)[pick, jsls[ri]]
        out[ri, :, :, ci, :] = vals.astype(np.float32).reshape(RPC, 5, CS)
    return out.reshape(N_ROIS, 5 * C)
